# revision 14
# baseline (speedup 1.0000x reference)
"""DenseSNN Trainium2 kernel: 4-layer LIF SNN, T=100 steps, B=128, D=H=2048, C=100.

Strategy
--------
The reference scans timesteps with all 4 layers inside the scan body, but the
dependency structure is feed-forward across layers: layer-l spikes at step t
depend only on layer-(l-1) spikes at steps <= t. So the computation unrolls into
per-layer phases:

    CUR1 = x @ W1 + b1          (batched over all T*B rows)
    S1   = LIF-scan_T(CUR1)     (elementwise in (B,H), sequential in T)
    CUR2 = S1 @ W2 + b2 ; S2 = LIF-scan(CUR2)
    CUR3 = S2 @ W3 + b3 ; S3 = LIF-scan(CUR3)
    CURo = S3 @ Wo + bo ; out = sum_t LIF-scan(CURo)

This turns the tiny per-step GEMMs into full-size GEMMs and makes pure
data-parallelism over batch (16 samples/core on 8 cores) communication-free.

On-chip layout is "transposed activations": [feature -> 16 chunks x 128
partitions, (t,b) -> free axis]. Weight-stationary matmuls (lhsT = W tile in
natural [D,H] layout) keep every tensor in this layout end to end; the host
pre-transposes x and re-assembles the output, so the device never transposes.

v3: GEMMs run in fp8 e4m3 DoubleRow (2 contraction chunks per matmul, fp32
PSUM); x and weights are host-cast to e4m3, spikes are exact in fp8. The LIF
scan tracks u = mem - 1 in bf16 so each step is  STT u = beta*u + cur*  (cur*
has beta-1 folded into the eviction bias),  TT u -= s_prev  (2x DVE mode), and
TS s = u > 0  (4x DVE mode); a scalar-engine copy casts each spike column
bf16 -> fp8 for the next layer's GEMM rhs, off the vector critical chain.
Output spikes are stored time-major and spike-count reduced with a log tree.
reset(t) == spike(t-1), which saves one compare per step.
"""

import numpy as np
import ml_dtypes

import concourse.bass as bass
import concourse.mybir as mybir
import concourse.tile as tile
from concourse import bacc
from concourse.bass_utils import run_bass_kernel_spmd

# Problem constants (hardcoded per contract)
T, B, D, H, C = 100, 128, 2048, 2048, 100
NCORES = 8
BC = B // NCORES          # 16 samples per core
R = T * BC                # 1600 rows (t,b) per core
KC = D // 128             # 16 contraction chunks
KP = KC // 2              # 8 contraction chunk-pairs (fp8 DoubleRow)
HC = H // 128             # 16 output-feature chunks
BETA = 0.9
NR = 512                  # row-slice width (multiple of BC, <= one PSUM bank)
SLICES = [(r0, min(NR, R - r0)) for r0 in range(0, R, NR)]
TP = 128                  # output spike stash: T padded to a power of two

F32 = mybir.dt.float32
BF16 = mybir.dt.bfloat16
FP8 = mybir.dt.float8e4
ALU = mybir.AluOpType
ACTF = mybir.ActivationFunctionType
DR = mybir.MatmulPerfMode.DoubleRow


def _build_nc():
    nc = bacc.Bacc("TRN2", target_bir_lowering=False)

    xT_d = nc.dram_tensor("xT", [KC, 128, R], FP8, kind="ExternalInput")
    w_d = [
        nc.dram_tensor("w1", [D, H], FP8, kind="ExternalInput"),
        nc.dram_tensor("w2", [D, H], FP8, kind="ExternalInput"),
        nc.dram_tensor("w3", [D, H], FP8, kind="ExternalInput"),
    ]
    wo_d = nc.dram_tensor("wo", [H, C], FP8, kind="ExternalInput")
    bias_d = nc.dram_tensor("biases", [128, 3 * HC], F32, kind="ExternalInput")
    bo_d = nc.dram_tensor("biaso", [C, 1], F32, kind="ExternalInput")
    out_d = nc.dram_tensor("out", [C, BC], F32, kind="ExternalOutput")

    with tile.TileContext(nc) as tc:
        with (
            tc.tile_pool(name="spool", bufs=2) as spool,
            tc.tile_pool(name="wpool", bufs=2) as wpool,
            tc.tile_pool(name="stream", bufs=2) as stream,
            tc.tile_pool(name="small", bufs=1) as small,
            tc.tile_pool(name="pspool", bufs=8, space="PSUM") as pspool,
        ):
            # Persistent fp8 spike tensors (rhs of the next layer's GEMM).
            S8 = [
                spool.tile([128, KC, R], FP8, tag="S", name=f"s8_{i}")
                for i in range(3)
            ]
            w_sb = [
                wpool.tile([128, KC, H], FP8, tag="W", name=f"w{i}_sb")
                for i in range(3)
            ]
            # chunk stride padded to 112 (DoubleRow ldweights needs step%16==0)
            CP = 112
            wo_sb = small.tile([128, KC, CP], FP8)

            # bf16 LIF state: u = mem - 1 (so spike test is u > 0), plus a
            # 2-step bf16 spike ring per layer feeding the reset subtract
            # and the scalar-engine fp8 cast.
            u_st = small.tile([128, 3, KC, BC], BF16)
            ring = small.tile([128, 3, 2, KC, BC], BF16)
            bias_sb = small.tile([128, 3 * HC], F32)

            # fp32 output-layer state
            sto = small.tile([128, 80], F32)
            memo = sto[:C, 0:BC]
            zo = sto[:C, BC:2 * BC]
            bo_sb = sto[:C, 4 * BC:4 * BC + 1]
            # output spikes, time-major [c, t, b]: contiguous per-step slots
            # and a contiguous halving tree for the final spike-count sum
            soT = small.tile([128, TP, BC], BF16)
            ssum = small.tile([128, BC], F32)

            nc.vector.memset(u_st[:], -1.0)
            nc.vector.memset(soT[:], 0.0)
            nc.vector.memset(sto[:], 0.0)
            nc.sync.dma_start(bias_sb, bias_d[:])
            nc.sync.dma_start(bo_sb, bo_d[:])
            for kc in range(KC):
                nc.sync.dma_start(
                    wo_sb[:, kc, :C], wo_d[kc * 128:(kc + 1) * 128, :]
                )

            def load_w(li):
                for kc in range(KC):
                    nc.sync.dma_start(
                        w_sb[li][:, kc, :], w_d[li][kc * 128:(kc + 1) * 128, :]
                    )

            load_w(0)
            load_w(1)

            def dense_layer(li, rhs_of, S_out3):
                """One hidden layer: fp8 DoubleRow matmuls + bf16 LIF scan.

                Software-pipelined: slice j's PSUM evictions are woven into
                the scan-step loop of slice j-1 so the scalar engine's FIFO
                alternates cast/evict instead of bursting 16 evictions ahead
                of the casts the vector scan is waiting on.
                """
                w3d = w_sb[li]
                u3 = u_st[:, li]
                r4 = ring[:, li]

                def emit_evict(j, cur, hc):
                    r0, nr = SLICES[j]
                    nc.scalar.activation(
                        cur[:, hc, :nr],
                        psl[j][hc][:, :nr],
                        ACTF.Identity,
                        bias=bias_sb[:, li * HC + hc: li * HC + hc + 1],
                        scale=1.0,
                    )

                def scan_slice(j, cur, ev=None):
                    """Scan slice j; interleave evictions of slice ev."""
                    r0, nr = SLICES[j]
                    pend = list(range(HC)) if ev is not None else []
                    for tl in range(nr // BC):
                        t = r0 // BC + tl
                        cur_t = cur[:, :, tl * BC:(tl + 1) * BC]
                        s_new = r4[:, t % 2]
                        # u = beta*u + cur*   (cur* has beta-1 folded in)
                        nc.vector.scalar_tensor_tensor(
                            u3, u3, BETA, cur_t, ALU.mult, ALU.add
                        )
                        if t > 0:
                            # reset-by-subtraction: u -= s_prev  (2x DVE)
                            nc.vector.tensor_tensor(
                                u3, u3, r4[:, (t + 1) % 2], ALU.subtract
                            )
                        # spike = u > 0   (4x DVE)
                        nc.vector.tensor_scalar(
                            s_new, u3, 0.0, None, ALU.is_gt
                        )
                        # fp8 cast for the next layer's GEMM rhs (scalar
                        # engine, off the vector critical chain)
                        nc.scalar.copy(
                            S_out3[:, :, t * BC:(t + 1) * BC], s_new
                        )
                        if pend and tl >= 4 and tl % 2 == 0:
                            emit_evict(ev, curs[ev], pend.pop(0))
                    for hc in pend:
                        emit_evict(ev, curs[ev], hc)

                curs = {}
                psl = {}
                prev = None
                for j, (r0, nr) in enumerate(SLICES):
                    rhs = rhs_of(r0, nr)
                    curs[j] = stream.tile(
                        [128, HC, NR], BF16, tag="cur", name="cur"
                    )
                    psl[j] = []
                    for hc in range(HC):
                        ps = pspool.tile([128, NR], F32, tag="ps", name="ps")
                        psl[j].append(ps)
                        for kp in range(KP):
                            nc.tensor.matmul(
                                ps[:, :nr],
                                w3d[:, 2 * kp:2 * kp + 2, hc * 128:(hc + 1) * 128],
                                rhs(kp),
                                start=(kp == 0),
                                stop=(kp == KP - 1),
                                perf_mode=DR,
                            )
                    if j == 0:
                        for hc in range(HC):
                            emit_evict(0, curs[0], hc)
                    else:
                        scan_slice(j - 1, curs[j - 1], ev=j)
                        del psl[j - 1], curs[j - 1]
                last = len(SLICES) - 1
                scan_slice(last, curs[last])

            # ---- Layer 1: rhs streamed from HBM (x^T fp8, host-pretransposed)
            def rhs_layer1(r0, nr):
                xin = stream.tile([128, KC, NR], FP8, tag="xin", name="xin")
                for kc in range(KC):
                    nc.sync.dma_start(
                        xin[:, kc, :nr], xT_d[kc][:, r0:r0 + nr]
                    )
                return lambda kp: xin[:, 2 * kp:2 * kp + 2, :nr]

            dense_layer(0, rhs_layer1, S8[0])

            # ---- Layers 2, 3: rhs from previous layer's fp8 spikes in SBUF
            def rhs_from(S_in3):
                def f(r0, nr):
                    return lambda kp: S_in3[:, 2 * kp:2 * kp + 2, r0:r0 + nr]
                return f

            load_w(2)
            dense_layer(1, rhs_from(S8[0]), S8[1])
            dense_layer(2, rhs_from(S8[1]), S8[2])

            # ---- Output layer: fp8 GEMM + classic LIF scan + tree reduce
            for r0, nr in SLICES:
                ps = pspool.tile([128, NR], F32, tag="ps", name="pso")
                for kp in range(KP):
                    nc.tensor.matmul(
                        ps[:C, :nr],
                        wo_sb[:, 2 * kp:2 * kp + 2, :C],
                        S8[2][:, 2 * kp:2 * kp + 2, r0:r0 + nr],
                        start=(kp == 0),
                        stop=(kp == KP - 1),
                        perf_mode=DR,
                    )
                curo = stream.tile([128, NR], F32, tag="curo", name="curo")
                curo_f = curo[:C, :nr]
                nc.scalar.activation(
                    curo_f, ps[:C, :nr], ACTF.Identity,
                    bias=bo_sb, scale=1.0,
                )
                for tl in range(nr // BC):
                    t = r0 // BC + tl
                    cur_t = curo_f[:, tl * BC:(tl + 1) * BC]
                    so_prev = zo if t == 0 else soT[:C, t - 1, :]
                    so_new = soT[:C, t, :]
                    nc.vector.scalar_tensor_tensor(
                        memo, memo, BETA, cur_t, ALU.mult, ALU.add
                    )
                    nc.vector.scalar_tensor_tensor(
                        so_new, memo, 1.0, so_prev, ALU.subtract, ALU.is_gt
                    )
                    nc.vector.tensor_tensor(memo, memo, so_prev, ALU.subtract)

            # spike count = halving tree over the (padded, zeroed) t axis
            soT_flat = soT.rearrange("p t b -> p (t b)")
            half = TP // 2
            while half >= 1:
                a = soT_flat[:C, 0:half * BC]
                b = soT_flat[:C, half * BC:2 * half * BC]
                if half == 1:
                    nc.vector.tensor_tensor(ssum[:C, :], a, b, ALU.add)
                else:
                    nc.vector.tensor_tensor(a, a, b, ALU.add)
                half //= 2
            nc.sync.dma_start(out_d[:], ssum[:C, :])

    nc.compile()
    return nc


_NC_CACHE = None


def _get_nc():
    global _NC_CACHE
    if _NC_CACHE is None:
        _NC_CACHE = _build_nc()
    return _NC_CACHE


def make_in_maps(x_seq, W1, b1, W2, b2, W3, b3, Wo, bo):
    f8 = ml_dtypes.float8_e4m3
    w1 = np.ascontiguousarray(W1.astype(f8))
    w2 = np.ascontiguousarray(W2.astype(f8))
    w3 = np.ascontiguousarray(W3.astype(f8))
    wo = np.ascontiguousarray(Wo.astype(f8))
    # beta-1 fold: the scan tracks u = mem - 1, so each step's current gets
    # the constant (beta - 1) added once, via the eviction bias.
    biases = np.concatenate(
        [b.reshape(HC, 128).T for b in (b1, b2, b3)], axis=1
    ).astype(np.float32) + np.float32(BETA - 1.0)         # [128, 48]
    biases = np.ascontiguousarray(biases)
    bo_a = np.ascontiguousarray(bo.reshape(C, 1).astype(np.float32))
    in_maps = []
    for c in range(NCORES):
        xs = x_seq[:, c * BC:(c + 1) * BC, :]              # [T, BC, D]
        xT = xs.transpose(2, 0, 1).reshape(KC, 128, R)     # [D,(t,b)] chunked
        in_maps.append({
            "xT": np.ascontiguousarray(xT.astype(f8)),
            "w1": w1, "w2": w2, "w3": w3, "wo": wo,
            "biases": biases, "biaso": bo_a,
        })
    return in_maps


def kernel(x_seq, W1, b1, W2, b2, W3, b3, Wo, bo):
    nc = _get_nc()
    in_maps = make_in_maps(x_seq, W1, b1, W2, b2, W3, b3, Wo, bo)
    res = run_bass_kernel_spmd(nc, in_maps, core_ids=list(range(NCORES)))
    outs = [res.results[c]["out"] for c in range(NCORES)]   # each [C, BC]
    return np.concatenate([o.T for o in outs], axis=0).astype(np.float32)


# revision 18
# speedup vs baseline: 1.0592x; 1.0592x over previous
"""DenseSNN Trainium2 kernel: 4-layer LIF SNN, T=100 steps, B=128, D=H=2048, C=100.

Strategy
--------
The reference scans timesteps with all 4 layers inside the scan body, but the
dependency structure is feed-forward across layers: layer-l spikes at step t
depend only on layer-(l-1) spikes at steps <= t. So the computation unrolls into
per-layer phases:

    CUR1 = x @ W1 + b1          (batched over all T*B rows)
    S1   = LIF-scan_T(CUR1)     (elementwise in (B,H), sequential in T)
    CUR2 = S1 @ W2 + b2 ; S2 = LIF-scan(CUR2)
    CUR3 = S2 @ W3 + b3 ; S3 = LIF-scan(CUR3)
    CURo = S3 @ Wo + bo ; out = sum_t LIF-scan(CURo)

This turns the tiny per-step GEMMs into full-size GEMMs and makes pure
data-parallelism over batch (16 samples/core on 8 cores) communication-free.

On-chip layout is "transposed activations": [feature -> 16 chunks x 128
partitions, (t,b) -> free axis]. Weight-stationary matmuls (lhsT = W tile in
natural [D,H] layout) keep every tensor in this layout end to end; the host
pre-transposes x and re-assembles the output, so the device never transposes.

v3: GEMMs run in fp8 e4m3 DoubleRow (2 contraction chunks per matmul, fp32
PSUM); x and weights are host-cast to e4m3, spikes are exact in fp8. The LIF
scan tracks u = mem - 1 in bf16 so each step is  STT u = beta*u + cur*  (cur*
has beta-1 folded into the eviction bias),  TT u -= s_prev  (2x DVE mode), and
TS s = u > 0  (4x DVE mode); a scalar-engine copy casts each spike column
bf16 -> fp8 for the next layer's GEMM rhs, off the vector critical chain.
Output spikes are stored time-major and spike-count reduced with a log tree.
reset(t) == spike(t-1), which saves one compare per step.
"""

import numpy as np
import ml_dtypes

import concourse.bass as bass
import concourse.mybir as mybir
import concourse.tile as tile
from concourse import bacc
from concourse.bass_utils import run_bass_kernel_spmd

# Problem constants (hardcoded per contract)
T, B, D, H, C = 100, 128, 2048, 2048, 100
NCORES = 8
BC = B // NCORES          # 16 samples per core
R = T * BC                # 1600 rows (t,b) per core
KC = D // 128             # 16 contraction chunks
KP = KC // 2              # 8 contraction chunk-pairs (fp8 DoubleRow)
HC = H // 128             # 16 output-feature chunks
BETA = 0.9
NR = 512                  # row-slice width (multiple of BC, <= one PSUM bank)
SLICES = [(r0, min(NR, R - r0)) for r0 in range(0, R, NR)]
TP = 128                  # output spike stash: T padded to a power of two

F32 = mybir.dt.float32
BF16 = mybir.dt.bfloat16
FP8 = mybir.dt.float8e4
ALU = mybir.AluOpType
ACTF = mybir.ActivationFunctionType
DR = mybir.MatmulPerfMode.DoubleRow


def _build_nc():
    nc = bacc.Bacc("TRN2", target_bir_lowering=False)

    xT_d = nc.dram_tensor("xT", [KC, 128, R], FP8, kind="ExternalInput")
    w_d = [
        nc.dram_tensor("w1", [D, H], FP8, kind="ExternalInput"),
        nc.dram_tensor("w2", [D, H], FP8, kind="ExternalInput"),
        nc.dram_tensor("w3", [D, H], FP8, kind="ExternalInput"),
    ]
    wo_d = nc.dram_tensor("wo", [H, C], FP8, kind="ExternalInput")
    bias_d = nc.dram_tensor("biases", [128, 3 * HC], F32, kind="ExternalInput")
    bo_d = nc.dram_tensor("biaso", [C, 1], F32, kind="ExternalInput")
    out_d = nc.dram_tensor("out", [C, BC], F32, kind="ExternalOutput")

    with tile.TileContext(nc) as tc:
        with (
            tc.tile_pool(name="spool", bufs=2) as spool,
            tc.tile_pool(name="wpool", bufs=2) as wpool,
            tc.tile_pool(name="stream", bufs=2) as stream,
            tc.tile_pool(name="small", bufs=1) as small,
            tc.tile_pool(name="pspool", bufs=8, space="PSUM") as pspool,
        ):
            # Persistent fp8 spike tensors (rhs of the next layer's GEMM).
            S8 = [
                spool.tile([128, KC, R], FP8, tag="S", name=f"s8_{i}")
                for i in range(3)
            ]
            w_sb = [
                wpool.tile([128, KC, H], FP8, tag="W", name=f"w{i}_sb")
                for i in range(3)
            ]
            # chunk stride padded to 112 (DoubleRow ldweights needs step%16==0)
            CP = 112
            wo_sb = small.tile([128, KC, CP], FP8)

            # bf16 LIF state: u = mem - 1 (so spike test is u > 0), plus a
            # 2-step bf16 spike ring per layer feeding the reset subtract
            # and the scalar-engine fp8 cast.
            u_st = small.tile([128, 3, KC, BC], BF16)
            NRING = 4
            ring = small.tile([128, 3, NRING, KC, BC], BF16)
            bias_sb = small.tile([128, 3 * HC], F32)

            # fp32 output-layer state
            sto = small.tile([128, 80], F32)
            memo = sto[:C, 0:BC]
            zo = sto[:C, BC:2 * BC]
            bo_sb = sto[:C, 4 * BC:4 * BC + 1]
            # output spikes, time-major [c, t, b]: contiguous per-step slots
            # and a contiguous halving tree for the final spike-count sum
            soT = small.tile([128, TP, BC], BF16)
            ssum = small.tile([128, BC], F32)

            nc.vector.memset(u_st[:], -1.0)
            nc.vector.memset(soT[:], 0.0)
            nc.vector.memset(sto[:], 0.0)
            nc.sync.dma_start(bias_sb, bias_d[:])
            nc.sync.dma_start(bo_sb, bo_d[:])

            # x slice 0 is DMA'd before the (4x bigger) weights so layer 1's
            # first GEMM starts as early as possible; w2/wo queue after w1.
            xin0 = stream.tile([128, KC, NR], FP8, tag="xin", name="xin")
            for kc in range(KC):
                nc.sync.dma_start(
                    xin0[:, kc, :SLICES[0][1]], xT_d[kc][:, 0:SLICES[0][1]]
                )

            def load_w(li):
                for kc in range(KC):
                    nc.sync.dma_start(
                        w_sb[li][:, kc, :], w_d[li][kc * 128:(kc + 1) * 128, :]
                    )

            load_w(0)
            load_w(1)
            for kc in range(KC):
                nc.sync.dma_start(
                    wo_sb[:, kc, :C], wo_d[kc * 128:(kc + 1) * 128, :]
                )

            def dense_layer(li, rhs_of, S_out3):
                """One hidden layer: fp8 DoubleRow matmuls + bf16 LIF scan.

                Software-pipelined: slice j's PSUM evictions are woven into
                the scan-step loop of slice j-1 so the scalar engine's FIFO
                alternates cast/evict instead of bursting 16 evictions ahead
                of the casts the vector scan is waiting on.
                """
                w3d = w_sb[li]
                u3 = u_st[:, li]
                r4 = ring[:, li]

                def emit_evict(j, cur, hc):
                    r0, nr = SLICES[j]
                    nc.scalar.activation(
                        cur[:, hc, :nr],
                        psl[j][hc][:, :nr],
                        ACTF.Identity,
                        bias=bias_sb[:, li * HC + hc: li * HC + hc + 1],
                        scale=1.0,
                    )

                def scan_slice(j, cur, ev=None):
                    """Scan slice j; interleave evictions of slice ev."""
                    r0, nr = SLICES[j]
                    pend = list(range(HC)) if ev is not None else []
                    for tl in range(nr // BC):
                        t = r0 // BC + tl
                        cur_t = cur[:, :, tl * BC:(tl + 1) * BC]
                        s_new = r4[:, t % NRING]
                        # u = beta*u + cur*   (cur* has beta-1 folded in)
                        nc.vector.scalar_tensor_tensor(
                            u3, u3, BETA, cur_t, ALU.mult, ALU.add
                        )
                        if t > 0:
                            # reset-by-subtraction: u -= s_prev  (2x DVE)
                            nc.vector.tensor_tensor(
                                u3, u3, r4[:, (t - 1) % NRING], ALU.subtract
                            )
                        # spike = u > 0   (4x DVE)
                        nc.vector.tensor_scalar(
                            s_new, u3, 0.0, None, ALU.is_gt
                        )
                        # fp8 cast for the next layer's GEMM rhs (scalar
                        # engine, off the vector critical chain)
                        nc.scalar.copy(
                            S_out3[:, :, t * BC:(t + 1) * BC], s_new
                        )
                        if pend and tl >= 4 and tl % 2 == 0:
                            emit_evict(ev, curs[ev], pend.pop(0))
                    for hc in pend:
                        emit_evict(ev, curs[ev], hc)

                curs = {}
                psl = {}
                prev = None
                for j, (r0, nr) in enumerate(SLICES):
                    rhs = rhs_of(r0, nr)
                    curs[j] = stream.tile(
                        [128, HC, NR], BF16, tag="cur", name="cur"
                    )
                    psl[j] = []
                    for hc in range(HC):
                        ps = pspool.tile([128, NR], F32, tag="ps", name="ps")
                        psl[j].append(ps)
                        for kp in range(KP):
                            nc.tensor.matmul(
                                ps[:, :nr],
                                w3d[:, 2 * kp:2 * kp + 2, hc * 128:(hc + 1) * 128],
                                rhs(kp),
                                start=(kp == 0),
                                stop=(kp == KP - 1),
                                perf_mode=DR,
                            )
                    if j == 0:
                        for hc in range(HC):
                            emit_evict(0, curs[0], hc)
                    else:
                        scan_slice(j - 1, curs[j - 1], ev=j)
                        del psl[j - 1], curs[j - 1]
                last = len(SLICES) - 1
                scan_slice(last, curs[last])

            # ---- Layer 1: rhs streamed from HBM (x^T fp8, host-pretransposed)
            def rhs_layer1(r0, nr):
                if r0 == 0:
                    xin = xin0
                else:
                    xin = stream.tile([128, KC, NR], FP8, tag="xin", name="xin")
                    for kc in range(KC):
                        nc.sync.dma_start(
                            xin[:, kc, :nr], xT_d[kc][:, r0:r0 + nr]
                        )
                return lambda kp: xin[:, 2 * kp:2 * kp + 2, :nr]

            dense_layer(0, rhs_layer1, S8[0])

            # ---- Layers 2, 3: rhs from previous layer's fp8 spikes in SBUF
            def rhs_from(S_in3):
                def f(r0, nr):
                    return lambda kp: S_in3[:, 2 * kp:2 * kp + 2, r0:r0 + nr]
                return f

            load_w(2)
            dense_layer(1, rhs_from(S8[0]), S8[1])
            dense_layer(2, rhs_from(S8[1]), S8[2])

            # ---- Output layer: fp8 GEMM + classic LIF scan + tree reduce
            for r0, nr in SLICES:
                ps = pspool.tile([128, NR], F32, tag="ps", name="pso")
                for kp in range(KP):
                    nc.tensor.matmul(
                        ps[:C, :nr],
                        wo_sb[:, 2 * kp:2 * kp + 2, :C],
                        S8[2][:, 2 * kp:2 * kp + 2, r0:r0 + nr],
                        start=(kp == 0),
                        stop=(kp == KP - 1),
                        perf_mode=DR,
                    )
                curo = stream.tile([128, NR], F32, tag="curo", name="curo")
                curo_f = curo[:C, :nr]
                nc.scalar.activation(
                    curo_f, ps[:C, :nr], ACTF.Identity,
                    bias=bo_sb, scale=1.0,
                )
                for tl in range(nr // BC):
                    t = r0 // BC + tl
                    cur_t = curo_f[:, tl * BC:(tl + 1) * BC]
                    so_prev = zo if t == 0 else soT[:C, t - 1, :]
                    so_new = soT[:C, t, :]
                    nc.vector.scalar_tensor_tensor(
                        memo, memo, BETA, cur_t, ALU.mult, ALU.add
                    )
                    nc.vector.scalar_tensor_tensor(
                        so_new, memo, 1.0, so_prev, ALU.subtract, ALU.is_gt
                    )
                    nc.vector.tensor_tensor(memo, memo, so_prev, ALU.subtract)

            # spike count = halving tree over the (padded, zeroed) t axis
            soT_flat = soT.rearrange("p t b -> p (t b)")
            half = TP // 2
            while half >= 1:
                a = soT_flat[:C, 0:half * BC]
                b = soT_flat[:C, half * BC:2 * half * BC]
                if half == 1:
                    nc.vector.tensor_tensor(ssum[:C, :], a, b, ALU.add)
                else:
                    nc.vector.tensor_tensor(a, a, b, ALU.add)
                half //= 2
            nc.sync.dma_start(out_d[:], ssum[:C, :])

    nc.compile()
    return nc


_NC_CACHE = None


def _get_nc():
    global _NC_CACHE
    if _NC_CACHE is None:
        _NC_CACHE = _build_nc()
    return _NC_CACHE


def make_in_maps(x_seq, W1, b1, W2, b2, W3, b3, Wo, bo):
    f8 = ml_dtypes.float8_e4m3
    w1 = np.ascontiguousarray(W1.astype(f8))
    w2 = np.ascontiguousarray(W2.astype(f8))
    w3 = np.ascontiguousarray(W3.astype(f8))
    wo = np.ascontiguousarray(Wo.astype(f8))
    # beta-1 fold: the scan tracks u = mem - 1, so each step's current gets
    # the constant (beta - 1) added once, via the eviction bias.
    biases = np.concatenate(
        [b.reshape(HC, 128).T for b in (b1, b2, b3)], axis=1
    ).astype(np.float32) + np.float32(BETA - 1.0)         # [128, 48]
    biases = np.ascontiguousarray(biases)
    bo_a = np.ascontiguousarray(bo.reshape(C, 1).astype(np.float32))
    in_maps = []
    for c in range(NCORES):
        xs = x_seq[:, c * BC:(c + 1) * BC, :]              # [T, BC, D]
        xT = xs.transpose(2, 0, 1).reshape(KC, 128, R)     # [D,(t,b)] chunked
        in_maps.append({
            "xT": np.ascontiguousarray(xT.astype(f8)),
            "w1": w1, "w2": w2, "w3": w3, "wo": wo,
            "biases": biases, "biaso": bo_a,
        })
    return in_maps


def kernel(x_seq, W1, b1, W2, b2, W3, b3, Wo, bo):
    nc = _get_nc()
    in_maps = make_in_maps(x_seq, W1, b1, W2, b2, W3, b3, Wo, bo)
    res = run_bass_kernel_spmd(nc, in_maps, core_ids=list(range(NCORES)))
    outs = [res.results[c]["out"] for c in range(NCORES)]   # each [C, BC]
    return np.concatenate([o.T for o in outs], axis=0).astype(np.float32)
